# revision 15
# baseline (speedup 1.0000x reference)
"""Trainium2 Bass kernel for DLRANet (4-layer low-rank MLP + log_softmax).

Strategy (v2):
- Data-parallel over 8 NeuronCores: each core computes 1024 rows of the
  8192-row batch; the low-rank factors K_i/Vt_i are replicated.
- Low-rank fused: h = z @ K and z' = relu(h @ Vt) computed per 128-wide
  w-chunk; activations stay feature-major ([feature, batch]) in SBUF.
- bc-sequential passes: each transition processes batch sub-chunk 0 as a
  full 32-w-chunk pass, then sub-chunk 1; trailing h-matmuls of a pass
  carry over into the next pass's slots (software pipelining, LAG chunks
  of slack for the relu to land), so the PE stream has no copy-latency
  bubbles at transition boundaries.
- HAM warm-up: the PE clock-gate starts at 1.2 GHz and needs ~3.4us of
  sustained busy to reach 2.4 GHz. Dummy matmuls on a zeroed SBUF tile
  run during the initial DMA wait so the real stream starts warm, and
  filler dummies pad the DMA-paced layer-0 phase so the PE never idles
  long enough to re-throttle.
- Final layer + log_softmax: logits for each 128-row chunk land in two
  [128,500] PSUM banks; exp (with sum accumulation) runs on ACT per
  half, the subtract runs on GpSimd (in-stream chunks) or DVE (tail
  chunks); chunks for batch sub-chunk 0 overlap the remaining t2 passes
  (t2 is split 512/256/256 so only the last 256 rows' softmax is tail).
- fp16 matmul datapath everywhere (1 row/cycle warm, same as bf16).
"""

import os
import numpy as np

_B, _DIN, _WID, _DOUT, _R = 8192, 1024, 4096, 1000, 128
_NC = 8
_BL = _B // _NC  # rows per core
_NB = 512  # batch sub-chunk
_DCH = _DIN // 128  # d-chunks in layer 0 (8)
_WCH = _WID // 128  # w-chunks per hidden layer (32)
_OH = 500  # output half width (2 x 500 = 1000)

_cache = {}


def build(reps=1):
    import concourse.bacc as bacc
    import concourse.mybir as mybir
    import concourse.tile as tile

    NWU = int(os.environ.get("KB_NWU", "12"))  # warm-up dummies
    LFILL = int(os.environ.get("KB_LFILL", "1"))  # fillers between L0 mms
    NFILL2 = int(os.environ.get("KB_NFILL2", "6"))  # fillers after L0
    LAG = int(os.environ.get("KB_LAG", "4"))  # h-mm lag, t0/t1
    LAG2 = int(os.environ.get("KB_LAG2", "8"))  # h-mm lag, t2
    T2ACT = int(os.environ.get("KB_T2ACT", "19"))  # ACT relus per 32 in t2

    F16 = mybir.dt.float16
    F32 = mybir.dt.float32
    AF = mybir.ActivationFunctionType

    nc = bacc.Bacc(trn_type="TRN2", target_bir_lowering=False, debug=False)

    # x layout host-side: [128, bc, c, 512] flattened (bc-major, chunk-major)
    xT_d = nc.dram_tensor("xT", [128, 2 * _DCH * _NB], F16, kind="ExternalInput").ap()
    k_d = [
        nc.dram_tensor(
            f"k{i}", [128, (_DCH if i == 0 else _WCH) * _R], F16, kind="ExternalInput"
        ).ap()
        for i in range(4)
    ]
    vt_d = [
        nc.dram_tensor(
            f"vt{i}", [128, _WID if i < 3 else _DOUT], F16, kind="ExternalInput"
        ).ap()
        for i in range(4)
    ]
    out_d = nc.dram_tensor("out", [_BL, _DOUT], F32, kind="ExternalOutput").ap()

    with tile.TileContext(nc) as tc:
        with tc.tile_pool(name="wp", bufs=1) as wp, tc.tile_pool(
            name="hp", bufs=1
        ) as hp, tc.tile_pool(name="zp", bufs=1) as zp, tc.tile_pool(
            name="fp", bufs=1
        ) as fp, tc.tile_pool(name="ps", bufs=1, space="PSUM") as ps:

            def body():
                # ---- warm-up tile (zeroed SBUF operand for dummy matmuls) ----
                wu_s = wp.tile([128, _NB], F16, tag="wu", name="wu")
                nc.vector.memset(wu_s[:], 0.0)
                wu_ps = ps.tile([128, _NB], F32, tag="wu", bufs=1, name="wups")

                def dummy_mm():
                    nc.tensor.matmul(
                        wu_ps[:], wu_s[:, 0:128], wu_s[:], start=True, stop=True
                    )

                # ---- DMA issues, need-ordered, spread across idle engines so
                # the issue stream isn't serialized on Sync ----
                k0_s = wp.tile([128, _DCH, _R], F16, tag="k0", name="k0")
                nc.sync.dma_start(
                    k0_s[:], k_d[0].rearrange("p (c r) -> p c r", c=_DCH)
                )
                xq = []  # 4 quarters: (bc0,c0-3),(bc0,c4-7),(bc1,c0-3),(bc1,c4-7)
                x_eng = [nc.gpsimd, nc.scalar, nc.vector, nc.sync]
                NXQ = 4 * _NB  # 2048 cols per quarter
                vt_q = [[None] * 4 for _ in range(4)]
                kn_q = [[None] * 4 for _ in range(3)]

                def load_xq(qi, eng):
                    xt = wp.tile([128, 4, _NB], F16, tag=f"x{qi}", name=f"x{qi}")
                    eng.dma_start(
                        xt[:],
                        xT_d[:, qi * NXQ : (qi + 1) * NXQ].rearrange(
                            "p (c b) -> p c b", c=4
                        ),
                    )
                    xq.append(xt)

                def load_vtq(i, q, eng):
                    w = _WID if i < 3 else _DOUT
                    qw = w // 4
                    v = wp.tile([128, qw], F16, tag=f"vt{i}q{q}", name=f"vt{i}q{q}")
                    eng.dma_start(v[:], vt_d[i][:, q * qw : (q + 1) * qw])
                    vt_q[i][q] = v

                def load_knq(i, q, eng):
                    k = wp.tile(
                        [128, _WCH // 4, _R], F16, tag=f"k{i+1}q{q}", name=f"k{i+1}q{q}"
                    )
                    eng.dma_start(
                        k[:],
                        k_d[i + 1][
                            :, q * (_WCH // 4) * _R : (q + 1) * (_WCH // 4) * _R
                        ].rearrange("p (c r) -> p c r", c=_WCH // 4),
                    )
                    kn_q[i][q] = k

                # front: what layer0 + transition-0-pass-A need, in need order,
                # spread across engines so issue isn't serialized on one queue
                load_xq(0, nc.gpsimd)
                load_xq(1, nc.scalar)
                load_vtq(0, 0, nc.sync)
                load_knq(0, 0, nc.gpsimd)
                load_vtq(0, 1, nc.scalar)
                load_xq(2, nc.sync)
                load_knq(0, 1, nc.gpsimd)
                load_xq(3, nc.scalar)
                load_vtq(0, 2, nc.gpsimd)
                load_knq(0, 2, nc.sync)
                load_knq(0, 3, nc.scalar)
                load_vtq(0, 3, nc.sync)
                vt3_s = wp.tile([128, _DOUT], F16, tag="vt3", name="vt3")

                # Later weights are issued from the GpSimd stream at points
                # that are semaphore-gated on mid-kernel results, so their
                # packets can't dilute the front-critical DMA bandwidth.
                def dma_group(i):
                    def go():
                        for q in range(4):
                            load_vtq(i, q, nc.gpsimd)
                            load_knq(i, q, nc.gpsimd)
                        if i == 2:
                            nc.gpsimd.dma_start(vt3_s[:], vt_d[3][:])

                    return go

                # ---- warm-up dummies (run during the DMA wait; HAM needs
                # ~3.4us of PE busy before it un-throttles the clock) ----
                for _ in range(NWU):
                    dummy_mm()

                # ---- engine helpers ----
                def copy_halves(dst, src, w, eng_a, eng_b):
                    h2 = w // 2
                    eng_a(dst[:, 0:h2], src[:, 0:h2])
                    eng_b(dst[:, h2:w], src[:, h2:w])

                def act_copy(d, s):
                    nc.scalar.copy(d, s)

                def dve_copy(d, s):
                    nc.vector.tensor_copy(d, s)

                # ---- layer 0, sub-chunk 0: h0[r, b] = K0^T @ x^T, DMA-paced
                # with dummy fillers so the PE stays busy (HAM) ----
                hacc0 = ps.tile([128, _NB], F32, tag="hacc", bufs=2, name="hacc_b0")
                for c in range(_DCH):
                    nc.tensor.matmul(
                        hacc0[:],
                        k0_s[:, c, :],
                        xq[c // 4][:, c % 4, :],
                        start=(c == 0),
                        stop=(c == _DCH - 1),
                    )
                    for _ in range(LFILL):
                        dummy_mm()
                h_cur = {}  # (bc) -> sbuf fp16 tile of current layer input
                h0b0 = hp.tile([128, _NB], F16, tag="h", bufs=4, name="h0_b0")
                copy_halves(h0b0, hacc0, _NB, act_copy, dve_copy)
                h_cur[0] = h0b0
                for _ in range(NFILL2):
                    dummy_mm()

                # layer 0, sub-chunk 1: emitted as extras inside t0 pass A
                hacc1 = ps.tile([128, _NB], F32, tag="hacc", bufs=2, name="hacc_b1")
                h0b1 = hp.tile([128, _NB], F16, tag="h", bufs=4, name="h0_b1")
                h_cur[1] = h0b1

                def l0_b1_op(c):
                    def op():
                        nc.tensor.matmul(
                            hacc1[:],
                            k0_s[:, c, :],
                            xq[2 + c // 4][:, c % 4, :],
                            start=(c == 0),
                            stop=(c == _DCH - 1),
                        )
                        if c == _DCH - 1:
                            copy_halves(h0b1, hacc1, _NB, act_copy, dve_copy)

                    return op

                l0b1_extras = [
                    ((13 + 2 * c) if c < 4 else (31 + 2 * (c - 4)), l0_b1_op(c))
                    for c in range(_DCH)
                ]

                # ---- final-layer chunk: 2 matmuls -> exp halves (ACT, accum)
                # -> ln -> subtract halves -> out DMA ----
                def emit_final_chunk(g, h3_tile, j, tail):
                    lhsT = h3_tile[:, j * 128 : (j + 1) * 128]
                    lg = []
                    for hh in range(2):
                        lgt = ps.tile(
                            [128, _NB], F32, tag="lg", bufs=2, name=f"lg{g}_{hh}"
                        )
                        nc.tensor.matmul(
                            lgt[:, 0:_OH],
                            lhsT,
                            vt3_s[:, hh * _OH : (hh + 1) * _OH],
                            start=True,
                            stop=True,
                        )
                        lg.append(lgt)

                    def softmax():
                        ss = []
                        for hh in range(2):
                            e_s = fp.tile(
                                [128, _OH], F16, tag="e", bufs=2, name=f"e{g}_{hh}"
                            )
                            ssum = fp.tile(
                                [128, 1], F32, tag="ss", bufs=4, name=f"ss{g}_{hh}"
                            )
                            nc.scalar.activation(
                                e_s[:], lg[hh][:, 0:_OH], AF.Exp, accum_out=ssum[:]
                            )
                            ss.append(ssum)
                        st = fp.tile([128, 1], F32, tag="st", bufs=2, name=f"st{g}")
                        nc.vector.tensor_scalar_add(st[:], ss[0][:], ss[1][:])
                        lns = fp.tile([128, 1], F32, tag="lns", bufs=2, name=f"lns{g}")
                        nc.scalar.activation(lns[:], st[:], AF.Ln)
                        o_s = fp.tile([128, _DOUT], F32, tag="os", bufs=3, name=f"os{g}")
                        sub = nc.vector.tensor_scalar_sub
                        for hh in range(2):
                            sub(
                                o_s[:, hh * _OH : (hh + 1) * _OH],
                                lg[hh][:, 0:_OH],
                                lns[:],
                            )
                        nc.sync.dma_start(out_d[g * 128 : (g + 1) * 128, :], o_s[:])

                    softmax()

                # ---- transition pass: one batch sub-chunk through one layer.
                # z-mm(wc) then, LAG chunks later, h-mm(wc-LAG); the last LAG
                # h-mms are returned as carry for the next pass's slots. ----
                relu_idx = [0]

                def emit_pass(t, h_in, w, hacc, lag, carry_in, extras, last=False):
                    extras = sorted(extras, key=lambda kv: kv[0])
                    pend = []
                    carry_q = list(carry_in)
                    slot = [0]

                    def fill_slot():
                        s = slot[0]
                        slot[0] += 1
                        if carry_q:
                            carry_q.pop(0)()
                        elif extras and s >= extras[0][0]:
                            extras.pop(0)[1]()

                    for wc in range(_WCH):
                        pz = ps.tile([128, _NB], F32, tag="pz", bufs=3, name=f"pz{t}_{wc}")
                        q, r = wc // (_WCH // 4), wc % (_WCH // 4)
                        nc.tensor.matmul(
                            pz[:, 0:w],
                            vt_q[t][q][:, r * 128 : (r + 1) * 128],
                            h_in[:, 0:w],
                            start=True,
                            stop=True,
                        )
                        fill_slot()
                        zt = zp.tile([128, _NB], F16, tag="zs", bufs=14, name=f"z{t}_{wc}")
                        if t == 2:
                            # DVE carries the softmax subtracts in this phase;
                            # give ACT the larger relu share (Bresenham spread)
                            on_act = (wc * T2ACT) // _WCH != ((wc + 1) * T2ACT) // _WCH
                        else:
                            ri = relu_idx[0]
                            relu_idx[0] += 1
                            on_act = ri % 2 == 0
                        if on_act:
                            nc.scalar.activation(zt[:, 0:w], pz[:, 0:w], AF.Relu)
                        else:
                            nc.vector.tensor_scalar_max(zt[:, 0:w], pz[:, 0:w], 0.0)
                        pend.append((wc, zt))
                        if wc >= lag:
                            wc2, zt2 = pend.pop(0)
                            q2, r2 = wc2 // (_WCH // 4), wc2 % (_WCH // 4)
                            nc.tensor.matmul(
                                hacc[:, 0:w],
                                kn_q[t][q2][:, r2, :],
                                zt2[:, 0:w],
                                start=(wc2 == 0),
                                stop=(wc2 == _WCH - 1),
                            )
                            fill_slot()

                    def h_op(wc2, zt2):
                        def op():
                            nc.tensor.matmul(
                                hacc[:, 0:w],
                                kn_q[t][wc2 // (_WCH // 4)][:, wc2 % (_WCH // 4), :],
                                zt2[:, 0:w],
                                start=(wc2 == 0),
                                stop=(wc2 == _WCH - 1),
                            )

                        return op

                    carry_out = [h_op(wc2, zt2) for wc2, zt2 in pend]
                    if last:
                        for op in carry_out:
                            op()
                        carry_out = []
                    # drain any unconsumed extras
                    for _, op in extras:
                        op()
                    return carry_out

                # ---- transitions 0 and 1: passes A (bc0) and B (bc1) ----
                carry = []
                for t in range(2):
                    for bc in range(2):
                        hacc = ps.tile(
                            [128, _NB], F32, tag="hacc", bufs=2, name=f"hacc{t+1}_{bc}"
                        )
                        h_nxt = hp.tile(
                            [128, _NB], F16, tag="h", bufs=4, name=f"h{t+1}_{bc}"
                        )
                        extras = l0b1_extras if (t == 0 and bc == 0) else []
                        carry = emit_pass(t, h_cur[bc], _NB, hacc, LAG, carry, extras)

                        def cp(h_nxt=h_nxt, hacc=hacc):
                            copy_halves(h_nxt, hacc, _NB, act_copy, dve_copy)

                        carry = carry + [cp]
                        if t == 0 and bc == 0:
                            carry = carry + [dma_group(1)]
                        elif t == 0 and bc == 1:
                            carry = carry + [dma_group(2)]
                        h_cur[bc] = h_nxt

                # ---- transition 2: sub-chunks 512 / 256 / 256; finals for
                # each sub-chunk overlap the following passes ----
                t2_parts = [(0, 0, _NB), (1, 0, 256), (1, 256, 512)]
                fin_extras = []  # final-chunk ops for the NEXT pass
                g_base = 0
                for pi, (bc, b0, b1) in enumerate(t2_parts):
                    w = b1 - b0
                    hacc = ps.tile(
                        [128, _NB], F32, tag="hacc", bufs=2, name=f"hacc3_{pi}"
                    )
                    h_in = h_cur[bc][:, b0:b1] if (b0, b1) != (0, _NB) else h_cur[bc]
                    last = pi == len(t2_parts) - 1
                    carry = emit_pass(
                        2, h_in, w, hacc, LAG2, carry, fin_extras, last=last
                    )
                    h3 = hp.tile([128, _NB], F16, tag="h3", bufs=2, name=f"h3_{pi}")

                    def cp3(h3=h3, hacc=hacc, w=w):
                        copy_halves(h3, hacc, w, act_copy, dve_copy)

                    nch = w // 128

                    if last:
                        cp3()
                        for j in range(nch):
                            emit_final_chunk(g_base + j, h3, j, tail=True)
                    else:
                        carry = carry + [cp3]

                        def fin_op(g, h3=h3, j=None):
                            def op():
                                emit_final_chunk(g, h3, j, tail=False)

                            return op

                        step = max(2, (2 * _WCH) // (nch + 1))
                        fin_extras = [
                            (4 + i * step, fin_op(g_base + j, h3, j))
                            for i, j in enumerate(range(nch))
                        ]
                    g_base += nch

            if reps == 1:
                body()
            else:
                with tc.For_i(0, reps):
                    body()

    # Pin all activation funcs (Relu/Copy/Exp/Ln) to one table set so the
    # whole kernel does a single ACT table load instead of thrashing.
    import concourse.bacc as bacc_mod
    from concourse.hw_specs import get_activation_tables as _real_tables

    def _pinned_tables(arch):
        tabs = _real_tables(arch)
        pinned = "natural_log_exp_and_others"
        if pinned in tabs:
            ours = tabs[pinned]
            tabs = {
                name: (funcs if name == pinned else (funcs - ours))
                for name, funcs in tabs.items()
            }
        return tabs

    bacc_mod.get_activation_tables = _pinned_tables
    try:
        nc.compile()
    finally:
        bacc_mod.get_activation_tables = _real_tables
    return nc


def _prep_inputs(x, K0, Vt0, K1, Vt1, K2, Vt2, K3, Vt3):
    """Host-side sharding + layout prep (fp16 cast, chunk-major weights,
    per-core bc-major transposed x shards)."""
    cast = lambda a: np.asarray(a, np.float32).astype(np.float16)

    def chunk_major(a, p=128):
        c = a.shape[0] // p
        return np.ascontiguousarray(
            a.reshape(c, p, a.shape[1]).transpose(1, 0, 2).reshape(p, c * a.shape[1])
        )

    ks = [chunk_major(cast(np.asarray(k, np.float32))) for k in (K0, K1, K2, K3)]
    vts = [cast(np.ascontiguousarray(v, np.float32)) for v in (Vt0, Vt1, Vt2, Vt3)]
    xr = cast(np.asarray(x, np.float32))
    in_maps = []
    for core in range(_NC):
        xs = xr[core * _BL : (core + 1) * _BL]  # [1024, 1024] batch x d
        # -> [128, bc, c, 512] flattened: feature-part-major, bc-major
        xT = np.ascontiguousarray(
            xs.T.reshape(_DCH, 128, 2, _NB).transpose(1, 2, 0, 3).reshape(128, -1)
        )
        m = {"xT": xT}
        for i in range(4):
            m[f"k{i}"] = ks[i]
            m[f"vt{i}"] = vts[i]
        in_maps.append(m)
    return in_maps


def kernel(x, K0, Vt0, K1, Vt1, K2, Vt2, K3, Vt3):
    from concourse import bass_utils

    if "nc" not in _cache:
        _cache["nc"] = build(reps=1)
    nc = _cache["nc"]
    in_maps = _prep_inputs(x, K0, Vt0, K1, Vt1, K2, Vt2, K3, Vt3)
    res = bass_utils.run_bass_kernel_spmd(nc, in_maps, core_ids=list(range(_NC)))
    return np.concatenate([r["out"] for r in res.results], axis=0)
